# revision 62
# baseline (speedup 1.0000x reference)
"""Trainium2 Bass kernel for nn_CombinedGNN (gnn_message_passing).

Strategy (8 NeuronCores, node/row parallel, zero collectives):
  - masks[1] in the reference is identically zero (elementwise pow of a 0/1
    matrix), so only mask0 = adj/rowdeg matters.
  - All T=12 timesteps' aggregations are mask0 @ data[t] -> batched into ONE
    accumulated matmul  X^T @ adjT  with X = data rearranged to [N, 96].
    The row 1/deg scaling uses a host-precomputed inverse-degree vector
    broadcast to 96 partitions via a tiny matmul emitted before phase 1.
  - adj is exactly representable in fp8e4 (0/1): the big matmul runs in
    fp8 DoubleRow perf mode (2 k-tiles per instruction, 0.5 cycles/row).
    X is split hi/lo into two fp8e4 operands (hi + lo == bf16-level
    precision); both accumulate into the same PSUM group.
  - Each core owns 625 nodes (padded to 640, two 320-node PSUM halves).
    adjT's column block streams in 10 fine-grained slices so matmuls
    start as soon as the first slice lands.
  - The sequential t-chain (his_prev/cur_prev recurrences) runs in
    [feature-on-partition, node-on-free] orientation with host-prepacked /
    permuted weight matrices so no on-chip transposes are needed.
"""

import numpy as np
import ml_dtypes

import concourse.bass as bass
import concourse.mybir as mybir
import concourse.bass_utils as bass_utils
from concourse.tile import TileContext
from concourse.vector_clock import ScopedClock
from contextlib import contextmanager


@contextmanager
def _lean_drain():
    """Skip end-of-kernel semaphore clears (one-shot NEFF; every
    run_bass_kernel_spmd call reloads the NEFF, which re-zeros sems)."""
    orig = TileContext._drain_and_barrier

    def patched(self, tick_clock, wait_clock):
        nc = self.nc
        drain_inst = nc.sync.drain()
        wait_clock.add_sem_waits(
            drain_inst.ins, ScopedClock({None: tick_clock.global_clock}))
        popped = nc._tile_sem_poison_stack.pop()
        assert popped is self._sem_poison

    TileContext._drain_and_barrier = patched
    try:
        yield
    finally:
        TileContext._drain_and_barrier = orig

# problem constants (hardcoded per harness contract)
N, T, DAY, L = 5000, 12, 8, 2
F = DAY - 1
DIM = T * DAY  # 96
NCORES = 8
NPC = N // NCORES        # 625 nodes per core
NP = 640                 # padded nodes per core
NH = NP // 2             # 320, node half processed per psum chunk
KT = 128                 # contraction tile (partitions; K padded to 5120)
NK = 5120                # padded contraction size
NKT = NK // KT           # 40 k-tiles
NKP = NKT // 2           # 20 DoubleRow k-pairs
NSL = 5                  # a DMA slices (4 k-pairs each, 5120B lines)
PSL = NKP // NSL         # 4 pairs per slice
XW = DIM                 # 96 (no ones column; inv-degree precomputed on host)

F32 = mybir.dt.float32
BF16 = mybir.dt.bfloat16
FP8 = mybir.dt.float8e4
BF16_NP = ml_dtypes.bfloat16
FP8_NP = ml_dtypes.float8_e4m3
DR = mybir.MatmulPerfMode.DoubleRow

_MAXW = 1


def split_multi_waits(nc):
    """Walrus in this container rejects instructions with >~2 sync waits.
    Hoist extra waits onto preceding single-wait NoOps on the same engine."""
    f = nc.m.functions[0]
    for bb in list(f.blocks):
        new, ctr = [], 0
        for inst in bb.instructions:
            si = inst.sync_info
            waits = list(si.on_wait) if (si and si.on_wait) else []
            if len(waits) > _MAXW:
                head, keep = waits[:-_MAXW], waits[-_MAXW:]
                for i in range(0, len(head), _MAXW):
                    nop = mybir.InstNoOp(
                        name=f"{inst.name}-wsplit{ctr}", engine=inst.engine,
                        ins=[], outs=[],
                        sync_info=mybir.SyncInfo(on_wait=head[i:i + _MAXW],
                                                 on_update=[]),
                    )
                    ctr += 1
                    new.append(nop)
                inst.sync_info = mybir.SyncInfo(
                    on_wait=keep,
                    on_update=list(si.on_update) if si.on_update else [])
            new.append(inst)
        bb.instructions = new


def build_nc():
    with _lean_drain():
        return _build_nc_inner()


def _build_nc_inner():
    nc = bass.Bass()
    # a[p, g, i, n] = adjT[(2g+i)*KT + p, n]  (fp8, 0/1 exact)
    a_d = nc.dram_tensor("a", [KT, NKP, 2, NP], FP8, kind="ExternalInput")
    # xe{h,l}[p, g, i, c] = Xhi/lo[(2g+i)*KT + p, c]
    xh_d = nc.dram_tensor("xh", [KT, NKP, 2, XW], FP8, kind="ExternalInput")
    xl_d = nc.dram_tensor("xl", [KT, NKP, 2, XW], FP8, kind="ExternalInput")
    rbi_d = nc.dram_tensor("rbi", [1, NP], BF16, kind="ExternalInput")
    dt_d = nc.dram_tensor("dt", [8, T, NP], BF16, kind="ExternalInput")
    pt_d = nc.dram_tensor("pt", [8, T, NP], BF16, kind="ExternalInput")
    # w1: [24, 96] — per t, rows 0:8 prev-block, 8:16 raw, 16:24 agg
    w1_d = nc.dram_tensor("w1", [24, DIM], BF16, kind="ExternalInput")
    # wcomb: [8, T, 104] — cols 0:96 wf block(t), cols 96:104 w2s block(t)
    wcomb_d = nc.dram_tensor("wcomb", [8, T * 104], BF16,
                             kind="ExternalInput")
    out_d = nc.dram_tensor("out", [DIM, NP], BF16, kind="ExternalOutput")

    with TileContext(nc) as tc:
        with (
            tc.tile_pool(name="const", bufs=1) as cpool,
            tc.tile_pool(name="pagg", bufs=2, space="PSUM") as pagg,
            tc.tile_pool(name="prb", bufs=1, space="PSUM") as prb,
            tc.tile_pool(name="pp1", bufs=3, space="PSUM") as pp1,
            tc.tile_pool(name="pcm", bufs=2, space="PSUM") as pcm,
        ):
            # Both HWDGE rings stream in parallel: tiny rbi leads SP (it
            # gates the rb matmuls at the head of the PE queue); a-slices
            # alternate across the rings in k order so phase-1 chases a
            # double-rate stream. xl rides mid-ACT (only the lo pass needs
            # it, by which time it has landed).
            rbi_t = cpool.tile([1, NP], BF16)
            nc.sync.dma_start(out=rbi_t, in_=rbi_d[:, :])
            # xh split: the first matmuls gate on just pairs 0-3 (98KB)
            # instead of the full 0.49MB, starting phase 1 ~4us earlier.
            xha_t = cpool.tile([KT, PSL, 2, XW], FP8)
            nc.sync.dma_start(out=xha_t, in_=xh_d[:, 0:PSL, :, :])
            xhb_t = cpool.tile([KT, NKP - PSL, 2, XW], FP8)
            w1_t = cpool.tile([24, DIM], BF16)
            nc.scalar.dma_start(out=w1_t, in_=w1_d[:, :])
            wcomb_t = cpool.tile([8, T * 104], BF16)
            nc.scalar.dma_start(out=wcomb_t, in_=wcomb_d[:, :])
            # dag: per-t matmul rhs [24, T, NP] — rows 0:8 prev state
            # (written by the chain relu), 8:16 dataT, 16:24 scaled agg
            dag_t = cpool.tile([24, T, NP], BF16)
            nc.scalar.dma_start(out=dag_t[8:16, :, :], in_=dt_d[:, :, :])
            nc.vector.memset(dag_t[0:8, 0, :], 0.0)
            # Ring balance: ACT runs at ~1/3 of SP's rate, so it carries only
            # xl (early, gates the first lo matmuls) and s3; the other four
            # a-slices ride SP. This drops the ACT ring's critical load from
            # 2.0MB (~27us, the real phase-1 limiter) to ~1.3MB.
            xl_t = cpool.tile([KT, NKP, 2, XW], FP8)
            nc.scalar.dma_start(out=xl_t, in_=xl_d[:, :, :, :])
            a_tiles = []
            slice_rings = [nc.sync, nc.sync, nc.sync, nc.scalar, nc.sync]
            for s in range(NSL):
                a_t = cpool.tile([KT, PSL, 2, NP], FP8, name=f"a{s}")
                slice_rings[s].dma_start(
                    out=a_t, in_=a_d[:, s * PSL:(s + 1) * PSL, :, :])
                a_tiles.append(a_t)
                if s == 0:
                    nc.sync.dma_start(out=xhb_t,
                                      in_=xh_d[:, PSL:NKP, :, :])
            pt_t = cpool.tile([8, T, NP], BF16)
            nc.scalar.dma_start(out=pt_t, in_=pt_d[:, :, :])

            ones_t = cpool.tile([1, DIM], BF16)
            nc.vector.memset(ones_t, 1.0)
            h2_t = cpool.tile([8, T, NP], BF16)
            aggs_t = cpool.tile([DIM, NP], BF16)
            outt_t = cpool.tile([DIM, NP], BF16)

            aggp_t = [pagg.tile([XW, NH], F32, tag="aggp", name=f"aggp{h}")
                      for h in range(2)]
            rb_t = [prb.tile([DIM, NH], F32, tag="rb", name=f"rb{h}")
                    for h in range(2)]
            pcombs = [pcm.tile([104, NH], F32, tag="pcm", name=f"pcomb{h}")
                      for h in range(2)]

            # inv-degree broadcast to 96 partitions — no phase-1 dependency,
            # emitted first so the PE queue clears it early. Copied to SBUF
            # via DVE max(x,0) (values positive; DVE reads at most one PSUM
            # operand per instruction, and the Scalar queue is busy issuing
            # DMAs at kernel start).
            rbs_t = cpool.tile([DIM, NP], BF16)
            for h in range(2):
                nc.tensor.matmul(rb_t[h], ones_t,
                                 rbi_t[:, h * NH:(h + 1) * NH],
                                 start=True, stop=True, skip_group_check=True)
                nc.vector.tensor_scalar_max(
                    rbs_t[:, h * NH:(h + 1) * NH], rb_t[h], 0.0)

            # phase 1: aggT[96, NH] += X{hi,lo}^T @ adjT_shard, fp8 DoubleRow
            # (2 k-tiles per mm). hi/lo interleave per SLICE so each slice's
            # lo matmuls run right after its hi ones — no lo-pass PE tail
            # after the stream ends.
            rings = [nc.sync, nc.gpsimd, nc.scalar]
            for s in range(NSL):
                a_t = a_tiles[s]
                for xi in range(2):
                    for j in range(PSL):
                        g = s * PSL + j
                        if xi == 1:
                            x_ap = xl_t[:, g, :, :]
                        elif s == 0:
                            x_ap = xha_t[:, j, :, :]
                        else:
                            x_ap = xhb_t[:, g - PSL, :, :]
                        for h in range(2):
                            nc.tensor.matmul(
                                aggp_t[h], x_ap,
                                a_t[:, j, :, h * NH:(h + 1) * NH],
                                start=(g == 0 and xi == 0),
                                stop=(g == NKP - 1 and xi == 1),
                                perf_mode=DR, skip_group_check=True)

            # transition: scale by inv-degree, then scatter per-t slabs into
            # dag full-width in t order. Slabs 0 and 1 get their own scale
            # ops so the chain's first steps start ~2us earlier; the rest
            # are scaled in one bulk op (DVE cost is free-dim-bound, so a
            # [80, x] op costs the same as an [8, x] one).
            def scale(r0, r1):
                for h in range(2):
                    cs = slice(h * NH, (h + 1) * NH)
                    nc.vector.tensor_mul(aggs_t[r0:r1, cs],
                                         aggp_t[h][r0:r1, :],
                                         rbs_t[r0:r1, cs])

            def scatter(t):
                rings[t % 3].dma_start(out=dag_t[16:24, t, :],
                                       in_=aggs_t[t * 8:(t + 1) * 8, :])

            scale(0, 32)
            scatter(0)
            scatter(1)
            scale(32, 64)
            scatter(2)
            scatter(3)
            scale(64, DIM)
            for t in range(4, T):
                scatter(t)

            # phase 2: sequential t-chain, both node halves interleaved.
            # Per (h, t): p1 = w1[t]^T @ dag[:, t]; h2 = relu(p1)+pos (DVE);
            # pcomb += wcomb[t]^T @ h2 (rows 0:96 final acc, 96:104 prev acc,
            # read mid-group by the prev relu). PE order p1(0), p1(1),
            # pcomb(0), pcomb(1) fills the p1->STT bubble with useful work;
            # prev relus split across ScalarE (h0) and DVE (h1).
            def chain_step(t):
                r8 = slice(t * 8, t * 8 + 8)
                p1s = []
                for h in range(2):
                    cs = slice(h * NH, (h + 1) * NH)
                    p1 = pp1.tile([8, NH], F32, tag="p1", name=f"p1_{h}_{t}")
                    nc.tensor.matmul(p1, w1_t[:, r8], dag_t[:, t, cs],
                                     start=True, stop=True)
                    p1s.append(p1)
                for h in range(2):
                    cs = slice(h * NH, (h + 1) * NH)
                    nc.vector.scalar_tensor_tensor(
                        h2_t[:, t, cs], p1s[h], 0.0, pt_t[:, t, cs],
                        op0=mybir.AluOpType.max, op1=mybir.AluOpType.add)
                for h in range(2):
                    cs = slice(h * NH, (h + 1) * NH)
                    nc.tensor.matmul(pcombs[h],
                                     wcomb_t[:, t * 104:(t + 1) * 104],
                                     h2_t[:, t, cs],
                                     start=(t == 0), stop=(t == T - 1),
                                     skip_group_check=True)
                if t < T - 1:
                    nc.scalar.activation(
                        dag_t[0:8, t + 1, 0:NH], pcombs[0][DIM:104, :],
                        mybir.ActivationFunctionType.Relu)
                    nc.vector.tensor_scalar_max(
                        dag_t[0:8, t + 1, NH:NP], pcombs[1][DIM:104, :], 0.0)

            def final(h, pcomb):
                cs = slice(h * NH, (h + 1) * NH)
                if h == 0:
                    nc.scalar.activation(outt_t[:, cs], pcomb[0:DIM, :],
                                         mybir.ActivationFunctionType.Relu)
                else:
                    nc.vector.tensor_scalar_max(outt_t[:, cs],
                                                pcomb[0:DIM, :], 0.0)
                rings[h].dma_start(out=out_d[:, cs], in_=outt_t[:, cs])

            for t in range(T):
                chain_step(t)
            final(0, pcombs[0])
            final(1, pcombs[1])

    split_multi_waits(nc)
    return nc


def prep_in_maps(adj, data, pos, his_W, cur_W, his_weight, cur_weight,
                 final_weight):
    adj = np.asarray(adj, dtype=np.float32)
    data = np.asarray(data, dtype=np.float32)
    pos = np.asarray(pos, dtype=np.float32)
    his_W = np.asarray(his_W, dtype=np.float32)
    cur_W = np.asarray(cur_W, dtype=np.float32)
    his_weight = np.asarray(his_weight, dtype=np.float32)
    cur_weight = np.asarray(cur_weight, dtype=np.float32)
    final_weight = np.asarray(final_weight, dtype=np.float32)

    # X = data rearranged [N, 96] (col = t*8+d); zero-padded to NK rows;
    # hi/lo fp8 split: hi + lo reproduces X to ~bf16 precision.
    X = np.ascontiguousarray(data.transpose(1, 0, 2).reshape(N, DIM))
    Xe = np.zeros((NK, XW), np.float32)
    Xe[:N, :] = X
    Xhi = Xe.astype(FP8_NP)
    Xlo = (Xe - Xhi.astype(np.float32)).astype(FP8_NP)
    # DoubleRow lhsT layout: [KT, NKP, 2, XW], [p, g, i, c] = row (2g+i)*KT+p
    def dr_pack(M):
        return np.ascontiguousarray(
            M.reshape(NKP, 2, KT, M.shape[1]).transpose(2, 0, 1, 3))
    xh_h = dr_pack(Xhi)
    xl_h = dr_pack(Xlo)

    adjT = np.ascontiguousarray(adj.T)
    inv_deg = 1.0 / np.maximum(adj.sum(axis=1), 1.0)  # [N]

    # weight packing (zero-padded block maps, see build_nc layout)
    # w1 [24, 96]: per-t lhsT for the merged p1 matmul over dag rows
    # [prev(8); raw(8); agg(8)]
    w1 = np.zeros((24, DIM), np.float32)
    for t in range(T):
        w1[0:7, t * 8:t * 8 + 7] = his_W[t][:, 21:28].T
        w1[7, t * 8 + 7] = cur_W[t][0, 3]
        w1[8:15, t * 8:t * 8 + 7] = his_W[t][:, 0:7].T
        w1[15, t * 8 + 7] = cur_W[t][0, 0]
        w1[16:23, t * 8:t * 8 + 7] = his_W[t][:, 7:14].T
        w1[23, t * 8 + 7] = cur_W[t][0, 1]
    # w2s[d, 8t'+o] = prev-update weight from h(t') feature d to output o
    w2 = np.zeros((8, DIM), np.float32)
    for tp in range(T):
        w2[0:7, tp * 8:tp * 8 + 7] = his_weight[:, 7 * tp:7 * tp + 7].T
        w2[7, tp * 8 + 7] = cur_weight[0, tp]
    # interleaved feature (8t+d) -> reference feature (7t+d | 84+t)
    f_ref = np.array([7 * t + d if d < 7 else 84 + t
                      for t in range(T) for d in range(8)])
    wf96 = final_weight[:, f_ref].T  # [96 (8t+d), 96 (out)]
    wf = np.ascontiguousarray(
        wf96.reshape(T, 8, DIM).transpose(1, 0, 2).reshape(8, T * DIM))
    # wcomb [8, T*104]: per t, cols 0:96 = wf block(t), cols 96:104 = w2s(t)
    wcomb = np.zeros((8, T, 104), np.float32)
    for t in range(T):
        wcomb[:, t, 0:DIM] = wf[:, t * DIM:(t + 1) * DIM]
        wcomb[:, t, DIM:104] = w2[:, t * 8:(t + 1) * 8]
    wcomb = np.ascontiguousarray(wcomb.reshape(8, T * 104))

    in_maps = []
    for c in range(NCORES):
        c0 = c * NPC
        ac = np.zeros((NK, NP), np.float32)
        ac[:N, :NPC] = adjT[:, c0:c0 + NPC]
        # a[p, g, i, n] = ac[(2g+i)*KT + p, n]
        ah = np.ascontiguousarray(
            ac.reshape(NKP, 2, KT, NP).transpose(2, 0, 1, 3)).astype(FP8_NP)
        rbi = np.zeros((1, NP), np.float32)
        rbi[0, :NPC] = inv_deg[c0:c0 + NPC]
        dtc = np.zeros((8, T, NP), np.float32)
        dtc[:, :, :NPC] = data[:, c0:c0 + NPC, :].transpose(2, 0, 1)
        ptc = np.zeros((8, T, NP), np.float32)
        ptc[:, :, :NPC] = pos[:, c0:c0 + NPC, :].transpose(2, 0, 1)
        in_maps.append({
            "a": ah, "xh": xh_h, "xl": xl_h,
            "rbi": rbi.astype(BF16_NP),
            "dt": dtc.astype(BF16_NP), "pt": ptc.astype(BF16_NP),
            "w1": w1.astype(BF16_NP), "wcomb": wcomb.astype(BF16_NP),
        })
    return in_maps


def assemble(results):
    out = np.empty((N, DIM), np.float32)
    for c in range(NCORES):
        out[c * NPC:(c + 1) * NPC, :] = \
            results[c]["out"][:, :NPC].T.astype(np.float32)
    return out


_NC_CACHE = None


def get_nc():
    global _NC_CACHE
    if _NC_CACHE is None:
        _NC_CACHE = build_nc()
    return _NC_CACHE


def run_spmd(in_maps, **kwargs):
    nc = get_nc()
    return bass_utils.run_bass_kernel_spmd(
        nc, in_maps, list(range(NCORES)), **kwargs)


def kernel(**inputs):
    in_maps = prep_in_maps(**inputs)
    res = run_spmd(in_maps)
    return assemble(res.results)


# revision 64
# speedup vs baseline: 1.0038x; 1.0038x over previous
"""Trainium2 Bass kernel for nn_CombinedGNN (gnn_message_passing).

Strategy (8 NeuronCores, node/row parallel, zero collectives):
  - masks[1] in the reference is identically zero (elementwise pow of a 0/1
    matrix), so only mask0 = adj/rowdeg matters.
  - All T=12 timesteps' aggregations are mask0 @ data[t] -> batched into ONE
    accumulated matmul  X^T @ adjT  with X = data rearranged to [N, 96].
    The row 1/deg scaling uses a host-precomputed inverse-degree vector
    broadcast to 96 partitions via a tiny matmul emitted before phase 1.
  - adj is exactly representable in fp8e4 (0/1): the big matmul runs in
    fp8 DoubleRow perf mode (2 k-tiles per instruction, 0.5 cycles/row).
    X is split hi/lo into two fp8e4 operands (hi + lo == bf16-level
    precision); both accumulate into the same PSUM group.
  - Each core owns 625 nodes (padded to 640, two 320-node PSUM halves).
    adjT's column block streams in 10 fine-grained slices so matmuls
    start as soon as the first slice lands.
  - The sequential t-chain (his_prev/cur_prev recurrences) runs in
    [feature-on-partition, node-on-free] orientation with host-prepacked /
    permuted weight matrices so no on-chip transposes are needed.
"""

import numpy as np
import ml_dtypes

import concourse.bass as bass
import concourse.mybir as mybir
import concourse.bass_utils as bass_utils
from concourse.tile import TileContext
from concourse.vector_clock import ScopedClock
from contextlib import contextmanager


@contextmanager
def _lean_drain():
    """Skip end-of-kernel semaphore clears (one-shot NEFF; every
    run_bass_kernel_spmd call reloads the NEFF, which re-zeros sems)."""
    orig = TileContext._drain_and_barrier

    def patched(self, tick_clock, wait_clock):
        nc = self.nc
        drain_inst = nc.sync.drain()
        wait_clock.add_sem_waits(
            drain_inst.ins, ScopedClock({None: tick_clock.global_clock}))
        popped = nc._tile_sem_poison_stack.pop()
        assert popped is self._sem_poison

    TileContext._drain_and_barrier = patched
    try:
        yield
    finally:
        TileContext._drain_and_barrier = orig

# problem constants (hardcoded per harness contract)
N, T, DAY, L = 5000, 12, 8, 2
F = DAY - 1
DIM = T * DAY  # 96
NCORES = 8
NPC = N // NCORES        # 625 nodes per core
NP = 640                 # padded nodes per core
NH = NP // 2             # 320, node half processed per psum chunk
KT = 128                 # contraction tile (partitions; K padded to 5120)
NK = 5120                # padded contraction size
NKT = NK // KT           # 40 k-tiles
NKP = NKT // 2           # 20 DoubleRow k-pairs
NSL = 5                  # a DMA slices (4 k-pairs each, 5120B lines)
PSL = NKP // NSL         # 4 pairs per slice
XW = DIM                 # 96 (no ones column; inv-degree precomputed on host)

F32 = mybir.dt.float32
BF16 = mybir.dt.bfloat16
FP8 = mybir.dt.float8e4
BF16_NP = ml_dtypes.bfloat16
FP8_NP = ml_dtypes.float8_e4m3
DR = mybir.MatmulPerfMode.DoubleRow

_MAXW = 1


def split_multi_waits(nc):
    """Walrus in this container rejects instructions with >~2 sync waits.
    Hoist extra waits onto preceding single-wait NoOps on the same engine."""
    f = nc.m.functions[0]
    for bb in list(f.blocks):
        new, ctr = [], 0
        for inst in bb.instructions:
            si = inst.sync_info
            waits = list(si.on_wait) if (si and si.on_wait) else []
            if len(waits) > _MAXW:
                head, keep = waits[:-_MAXW], waits[-_MAXW:]
                for i in range(0, len(head), _MAXW):
                    nop = mybir.InstNoOp(
                        name=f"{inst.name}-wsplit{ctr}", engine=inst.engine,
                        ins=[], outs=[],
                        sync_info=mybir.SyncInfo(on_wait=head[i:i + _MAXW],
                                                 on_update=[]),
                    )
                    ctr += 1
                    new.append(nop)
                inst.sync_info = mybir.SyncInfo(
                    on_wait=keep,
                    on_update=list(si.on_update) if si.on_update else [])
            new.append(inst)
        bb.instructions = new


def build_nc():
    with _lean_drain():
        return _build_nc_inner()


def _build_nc_inner():
    nc = bass.Bass()
    # a[p, g, i, n] = adjT[(2g+i)*KT + p, n]  (fp8, 0/1 exact)
    a_d = nc.dram_tensor("a", [KT, NKP, 2, NP], FP8, kind="ExternalInput")
    # xe{h,l}[p, g, i, c] = Xhi/lo[(2g+i)*KT + p, c]
    xh_d = nc.dram_tensor("xh", [KT, NKP, 2, XW], FP8, kind="ExternalInput")
    xl_d = nc.dram_tensor("xl", [KT, NKP, 2, XW], FP8, kind="ExternalInput")
    rbi_d = nc.dram_tensor("rbi", [1, NP], BF16, kind="ExternalInput")
    dt_d = nc.dram_tensor("dt", [8, T, NP], BF16, kind="ExternalInput")
    pt_d = nc.dram_tensor("pt", [8, T, NP], BF16, kind="ExternalInput")
    # w1: [24, 96] — per t, rows 0:8 prev-block, 8:16 raw, 16:24 agg
    w1_d = nc.dram_tensor("w1", [24, DIM], BF16, kind="ExternalInput")
    # wcomb: [8, T, 104] — cols 0:96 wf block(t), cols 96:104 w2s block(t)
    wcomb_d = nc.dram_tensor("wcomb", [8, T * 104], BF16,
                             kind="ExternalInput")
    out_d = nc.dram_tensor("out", [DIM, NP], BF16, kind="ExternalOutput")

    with TileContext(nc) as tc:
        with (
            tc.tile_pool(name="const", bufs=1) as cpool,
            tc.tile_pool(name="pagg", bufs=2, space="PSUM") as pagg,
            tc.tile_pool(name="prb", bufs=1, space="PSUM") as prb,
            tc.tile_pool(name="pp1", bufs=3, space="PSUM") as pp1,
            tc.tile_pool(name="pcm", bufs=2, space="PSUM") as pcm,
        ):
            # Both HWDGE rings stream in parallel: tiny rbi leads SP (it
            # gates the rb matmuls at the head of the PE queue); a-slices
            # alternate across the rings in k order so phase-1 chases a
            # double-rate stream. xl rides mid-ACT (only the lo pass needs
            # it, by which time it has landed).
            rbi_t = cpool.tile([1, NP], BF16)
            nc.sync.dma_start(out=rbi_t, in_=rbi_d[:, :])
            # xh split: the first matmuls gate on just pairs 0-3 (98KB)
            # instead of the full 0.49MB, starting phase 1 ~4us earlier.
            xha_t = cpool.tile([KT, PSL, 2, XW], FP8)
            nc.sync.dma_start(out=xha_t, in_=xh_d[:, 0:PSL, :, :])
            xhb_t = cpool.tile([KT, NKP - PSL, 2, XW], FP8)
            w1_t = cpool.tile([24, DIM], BF16)
            nc.scalar.dma_start(out=w1_t, in_=w1_d[:, :])
            wcomb_t = cpool.tile([8, T * 104], BF16)
            nc.scalar.dma_start(out=wcomb_t, in_=wcomb_d[:, :])
            # dag: per-t matmul rhs [24, T, NP] — rows 0:8 prev state
            # (written by the chain relu), 8:16 dataT, 16:24 scaled agg
            dag_t = cpool.tile([24, T, NP], BF16)
            nc.scalar.dma_start(out=dag_t[8:16, :, :], in_=dt_d[:, :, :])
            nc.vector.memset(dag_t[0:8, 0, :], 0.0)
            # Ring balance: ACT runs at ~1/3 of SP's rate, so it carries only
            # xl (early, gates the first lo matmuls) and s3; the other four
            # a-slices ride SP. This drops the ACT ring's critical load from
            # 2.0MB (~27us, the real phase-1 limiter) to ~1.3MB.
            xl_t = cpool.tile([KT, NKP, 2, XW], FP8)
            nc.scalar.dma_start(out=xl_t, in_=xl_d[:, :, :, :])
            a_tiles = []
            slice_rings = [nc.sync, nc.sync, nc.sync, nc.scalar, nc.sync]
            for s in range(NSL):
                a_t = cpool.tile([KT, PSL, 2, NP], FP8, name=f"a{s}")
                slice_rings[s].dma_start(
                    out=a_t, in_=a_d[:, s * PSL:(s + 1) * PSL, :, :])
                a_tiles.append(a_t)
                if s == 0:
                    nc.sync.dma_start(out=xhb_t,
                                      in_=xh_d[:, PSL:NKP, :, :])
            pt_t = cpool.tile([8, T, NP], BF16)
            nc.scalar.dma_start(out=pt_t, in_=pt_d[:, :, :])

            ones_t = cpool.tile([1, DIM], BF16)
            nc.vector.memset(ones_t, 1.0)
            h2_t = cpool.tile([8, T, NP], BF16)
            aggs_t = cpool.tile([DIM, NP], BF16)
            outt_t = cpool.tile([DIM, NP], BF16)

            aggp_t = [pagg.tile([XW, NH], F32, tag="aggp", name=f"aggp{h}")
                      for h in range(2)]
            rb_t = [prb.tile([DIM, NH], F32, tag="rb", name=f"rb{h}")
                    for h in range(2)]
            pcombs = [pcm.tile([104, NH], F32, tag="pcm", name=f"pcomb{h}")
                      for h in range(2)]

            # inv-degree broadcast to 96 partitions — no phase-1 dependency,
            # emitted first so the PE queue clears it early. Copied to SBUF
            # via DVE max(x,0) (values positive; DVE reads at most one PSUM
            # operand per instruction, and the Scalar queue is busy issuing
            # DMAs at kernel start).
            rbs_t = cpool.tile([DIM, NP], BF16)
            for h in range(2):
                nc.tensor.matmul(rb_t[h], ones_t,
                                 rbi_t[:, h * NH:(h + 1) * NH],
                                 start=True, stop=True, skip_group_check=True)
                nc.vector.tensor_scalar_max(
                    rbs_t[:, h * NH:(h + 1) * NH], rb_t[h], 0.0)

            # phase 1: aggT[96, NH] += X{hi,lo}^T @ adjT_shard, fp8 DoubleRow
            # (2 k-tiles per mm). hi/lo interleave per SLICE so each slice's
            # lo matmuls run right after its hi ones — no lo-pass PE tail
            # after the stream ends.
            rings = [nc.sync, nc.gpsimd, nc.scalar]
            for s in range(NSL):
                a_t = a_tiles[s]
                for xi in range(2):
                    for j in range(PSL):
                        g = s * PSL + j
                        if xi == 1:
                            x_ap = xl_t[:, g, :, :]
                        elif s == 0:
                            x_ap = xha_t[:, j, :, :]
                        else:
                            x_ap = xhb_t[:, g - PSL, :, :]
                        for h in range(2):
                            nc.tensor.matmul(
                                aggp_t[h], x_ap,
                                a_t[:, j, :, h * NH:(h + 1) * NH],
                                start=(g == 0 and xi == 0),
                                stop=(g == NKP - 1 and xi == 1),
                                perf_mode=DR, skip_group_check=True)

            # transition: scale by inv-degree, then scatter per-t slabs into
            # dag full-width in t order. Slabs 0 and 1 get their own scale
            # ops so the chain's first steps start ~2us earlier; the rest
            # are scaled in one bulk op (DVE cost is free-dim-bound, so a
            # [80, x] op costs the same as an [8, x] one).
            def scale(r0, r1):
                for h in range(2):
                    cs = slice(h * NH, (h + 1) * NH)
                    nc.vector.tensor_mul(aggs_t[r0:r1, cs],
                                         aggp_t[h][r0:r1, :],
                                         rbs_t[r0:r1, cs])

            def scatter(t):
                rings[t % 3].dma_start(out=dag_t[16:24, t, :],
                                       in_=aggs_t[t * 8:(t + 1) * 8, :])

            scale(0, 32)
            scatter(0)
            scatter(1)
            scale(32, 64)
            scatter(2)
            scatter(3)
            scale(64, DIM)
            for t in range(4, T):
                scatter(t)

            # phase 2: sequential t-chain, both node halves interleaved.
            # Per (h, t): p1 = w1[t]^T @ dag[:, t]; h2 = relu(p1)+pos (DVE);
            # pcomb += wcomb[t]^T @ h2 (rows 0:96 final acc, 96:104 prev acc,
            # read mid-group by the prev relu). PE order p1(0), p1(1),
            # pcomb(0), pcomb(1) fills the p1->STT bubble with useful work;
            # prev relus split across ScalarE (h0) and DVE (h1).
            def chain_step(t):
                r8 = slice(t * 8, t * 8 + 8)
                p1s = []
                for h in range(2):
                    cs = slice(h * NH, (h + 1) * NH)
                    p1 = pp1.tile([8, NH], F32, tag="p1", name=f"p1_{h}_{t}")
                    nc.tensor.matmul(p1, w1_t[:, r8], dag_t[:, t, cs],
                                     start=True, stop=True)
                    p1s.append(p1)
                for h in range(2):
                    cs = slice(h * NH, (h + 1) * NH)
                    nc.vector.scalar_tensor_tensor(
                        h2_t[:, t, cs], p1s[h], 0.0, pt_t[:, t, cs],
                        op0=mybir.AluOpType.max, op1=mybir.AluOpType.add)
                for h in range(2):
                    cs = slice(h * NH, (h + 1) * NH)
                    nc.tensor.matmul(pcombs[h],
                                     wcomb_t[:, t * 104:(t + 1) * 104],
                                     h2_t[:, t, cs],
                                     start=(t == 0), stop=(t == T - 1),
                                     skip_group_check=True)
                if t < T - 1:
                    nc.scalar.activation(
                        dag_t[0:8, t + 1, 0:NH], pcombs[0][DIM:104, :],
                        mybir.ActivationFunctionType.Relu)
                    nc.vector.tensor_scalar_max(
                        dag_t[0:8, t + 1, NH:NP], pcombs[1][DIM:104, :], 0.0)

            def final(h, pcomb):
                cs = slice(h * NH, (h + 1) * NH)
                if h == 0:
                    nc.scalar.activation(outt_t[:, cs], pcomb[0:DIM, :],
                                         mybir.ActivationFunctionType.Relu)
                else:
                    nc.vector.tensor_scalar_max(outt_t[:, cs],
                                                pcomb[0:DIM, :], 0.0)
                rings[h].dma_start(out=out_d[:, cs], in_=outt_t[:, cs])

            for t in range(T):
                chain_step(t)
            final(0, pcombs[0])
            final(1, pcombs[1])

    split_multi_waits(nc)
    return nc


def prep_in_maps(adj, data, pos, his_W, cur_W, his_weight, cur_weight,
                 final_weight):
    adj = np.asarray(adj, dtype=np.float32)
    data = np.asarray(data, dtype=np.float32)
    pos = np.asarray(pos, dtype=np.float32)
    his_W = np.asarray(his_W, dtype=np.float32)
    cur_W = np.asarray(cur_W, dtype=np.float32)
    his_weight = np.asarray(his_weight, dtype=np.float32)
    cur_weight = np.asarray(cur_weight, dtype=np.float32)
    final_weight = np.asarray(final_weight, dtype=np.float32)

    # X = data rearranged [N, 96] (col = t*8+d); zero-padded to NK rows;
    # hi/lo fp8 split: hi + lo reproduces X to ~bf16 precision.
    X = np.ascontiguousarray(data.transpose(1, 0, 2).reshape(N, DIM))
    Xe = np.zeros((NK, XW), np.float32)
    Xe[:N, :] = X
    Xhi = Xe.astype(FP8_NP)
    Xlo = (Xe - Xhi.astype(np.float32)).astype(FP8_NP)
    # DoubleRow lhsT layout: [KT, NKP, 2, XW], [p, g, i, c] = row (2g+i)*KT+p
    def dr_pack(M):
        return np.ascontiguousarray(
            M.reshape(NKP, 2, KT, M.shape[1]).transpose(2, 0, 1, 3))
    xh_h = dr_pack(Xhi)
    xl_h = dr_pack(Xlo)

    adjT = np.ascontiguousarray(adj.T)
    inv_deg = 1.0 / np.maximum(adj.sum(axis=1), 1.0)  # [N]

    # weight packing (zero-padded block maps, see build_nc layout)
    # w1 [24, 96]: per-t lhsT for the merged p1 matmul over dag rows
    # [prev(8); raw(8); agg(8)]
    w1 = np.zeros((24, DIM), np.float32)
    for t in range(T):
        w1[0:7, t * 8:t * 8 + 7] = his_W[t][:, 21:28].T
        w1[7, t * 8 + 7] = cur_W[t][0, 3]
        w1[8:15, t * 8:t * 8 + 7] = his_W[t][:, 0:7].T
        w1[15, t * 8 + 7] = cur_W[t][0, 0]
        w1[16:23, t * 8:t * 8 + 7] = his_W[t][:, 7:14].T
        w1[23, t * 8 + 7] = cur_W[t][0, 1]
    # w2s[d, 8t'+o] = prev-update weight from h(t') feature d to output o
    w2 = np.zeros((8, DIM), np.float32)
    for tp in range(T):
        w2[0:7, tp * 8:tp * 8 + 7] = his_weight[:, 7 * tp:7 * tp + 7].T
        w2[7, tp * 8 + 7] = cur_weight[0, tp]
    # interleaved feature (8t+d) -> reference feature (7t+d | 84+t)
    f_ref = np.array([7 * t + d if d < 7 else 84 + t
                      for t in range(T) for d in range(8)])
    wf96 = final_weight[:, f_ref].T  # [96 (8t+d), 96 (out)]
    wf = np.ascontiguousarray(
        wf96.reshape(T, 8, DIM).transpose(1, 0, 2).reshape(8, T * DIM))
    # wcomb [8, T*104]: per t, cols 0:96 = wf block(t), cols 96:104 = w2s(t)
    wcomb = np.zeros((8, T, 104), np.float32)
    for t in range(T):
        wcomb[:, t, 0:DIM] = wf[:, t * DIM:(t + 1) * DIM]
        wcomb[:, t, DIM:104] = w2[:, t * 8:(t + 1) * 8]
    wcomb = np.ascontiguousarray(wcomb.reshape(8, T * 104))

    in_maps = []
    for c in range(NCORES):
        c0 = c * NPC
        ac = np.zeros((NK, NP), np.float32)
        ac[:N, :NPC] = adjT[:, c0:c0 + NPC]
        # a[p, g, i, n] = ac[(2g+i)*KT + p, n]
        ah = np.ascontiguousarray(
            ac.reshape(NKP, 2, KT, NP).transpose(2, 0, 1, 3)).astype(FP8_NP)
        rbi = np.zeros((1, NP), np.float32)
        rbi[0, :NPC] = inv_deg[c0:c0 + NPC]
        dtc = np.zeros((8, T, NP), np.float32)
        dtc[:, :, :NPC] = data[:, c0:c0 + NPC, :].transpose(2, 0, 1)
        ptc = np.zeros((8, T, NP), np.float32)
        ptc[:, :, :NPC] = pos[:, c0:c0 + NPC, :].transpose(2, 0, 1)
        in_maps.append({
            "a": ah, "xh": xh_h, "xl": xl_h,
            "rbi": rbi.astype(BF16_NP),
            "dt": dtc.astype(BF16_NP), "pt": ptc.astype(BF16_NP),
            "w1": w1.astype(BF16_NP), "wcomb": wcomb.astype(BF16_NP),
        })
    return in_maps


def assemble(results):
    out = np.empty((N, DIM), np.float32)
    for c in range(NCORES):
        out[c * NPC:(c + 1) * NPC, :] = \
            results[c]["out"][:, :NPC].T.astype(np.float32)
    return out


_NC_CACHE = None


def get_nc():
    global _NC_CACHE
    if _NC_CACHE is None:
        _NC_CACHE = build_nc()
    return _NC_CACHE


def run_spmd(in_maps, **kwargs):
    nc = get_nc()
    return bass_utils.run_bass_kernel_spmd(
        nc, in_maps, list(range(NCORES)), **kwargs)


def kernel(**inputs):
    in_maps = prep_in_maps(**inputs)
    res = run_spmd(in_maps)
    return assemble(res.results)


# revision 65
# speedup vs baseline: 1.0052x; 1.0013x over previous
"""Trainium2 Bass kernel for nn_CombinedGNN (gnn_message_passing).

Strategy (8 NeuronCores, node/row parallel, zero collectives):
  - masks[1] in the reference is identically zero (elementwise pow of a 0/1
    matrix), so only mask0 = adj/rowdeg matters.
  - All T=12 timesteps' aggregations are mask0 @ data[t] -> batched into ONE
    accumulated matmul  X^T @ adjT  with X = data rearranged to [N, 96].
    The row 1/deg scaling uses a host-precomputed inverse-degree vector
    broadcast to 96 partitions via a tiny matmul emitted before phase 1.
  - adj is exactly representable in fp8e4 (0/1): the big matmul runs in
    fp8 DoubleRow perf mode (2 k-tiles per instruction, 0.5 cycles/row).
    X is split hi/lo into two fp8e4 operands (hi + lo == bf16-level
    precision); both accumulate into the same PSUM group.
  - Each core owns 625 nodes (padded to 640, two 320-node PSUM halves).
    adjT's column block streams in 10 fine-grained slices so matmuls
    start as soon as the first slice lands.
  - The sequential t-chain (his_prev/cur_prev recurrences) runs in
    [feature-on-partition, node-on-free] orientation with host-prepacked /
    permuted weight matrices so no on-chip transposes are needed.
"""

import numpy as np
import ml_dtypes

import concourse.bass as bass
import concourse.mybir as mybir
import concourse.bass_utils as bass_utils
from concourse.tile import TileContext
from concourse.vector_clock import ScopedClock
from contextlib import contextmanager


@contextmanager
def _lean_drain():
    """Skip end-of-kernel semaphore clears (one-shot NEFF; every
    run_bass_kernel_spmd call reloads the NEFF, which re-zeros sems)."""
    orig = TileContext._drain_and_barrier

    def patched(self, tick_clock, wait_clock):
        nc = self.nc
        drain_inst = nc.sync.drain()
        wait_clock.add_sem_waits(
            drain_inst.ins, ScopedClock({None: tick_clock.global_clock}))
        popped = nc._tile_sem_poison_stack.pop()
        assert popped is self._sem_poison

    TileContext._drain_and_barrier = patched
    try:
        yield
    finally:
        TileContext._drain_and_barrier = orig

# problem constants (hardcoded per harness contract)
N, T, DAY, L = 5000, 12, 8, 2
F = DAY - 1
DIM = T * DAY  # 96
NCORES = 8
NPC = N // NCORES        # 625 nodes per core
NP = 640                 # padded nodes per core
NH = NP // 2             # 320, node half processed per psum chunk
KT = 128                 # contraction tile (partitions; K padded to 5120)
NK = 5120                # padded contraction size
NKT = NK // KT           # 40 k-tiles
NKP = NKT // 2           # 20 DoubleRow k-pairs
NSL = 5                  # a DMA slices (4 k-pairs each, 5120B lines)
PSL = NKP // NSL         # 4 pairs per slice
XW = DIM                 # 96 (no ones column; inv-degree precomputed on host)

F32 = mybir.dt.float32
BF16 = mybir.dt.bfloat16
FP8 = mybir.dt.float8e4
BF16_NP = ml_dtypes.bfloat16
FP8_NP = ml_dtypes.float8_e4m3
DR = mybir.MatmulPerfMode.DoubleRow

_MAXW = 1


def split_multi_waits(nc):
    """Walrus in this container rejects instructions with >~2 sync waits.
    Hoist extra waits onto preceding single-wait NoOps on the same engine."""
    f = nc.m.functions[0]
    for bb in list(f.blocks):
        new, ctr = [], 0
        for inst in bb.instructions:
            si = inst.sync_info
            waits = list(si.on_wait) if (si and si.on_wait) else []
            if len(waits) > _MAXW:
                head, keep = waits[:-_MAXW], waits[-_MAXW:]
                for i in range(0, len(head), _MAXW):
                    nop = mybir.InstNoOp(
                        name=f"{inst.name}-wsplit{ctr}", engine=inst.engine,
                        ins=[], outs=[],
                        sync_info=mybir.SyncInfo(on_wait=head[i:i + _MAXW],
                                                 on_update=[]),
                    )
                    ctr += 1
                    new.append(nop)
                inst.sync_info = mybir.SyncInfo(
                    on_wait=keep,
                    on_update=list(si.on_update) if si.on_update else [])
            new.append(inst)
        bb.instructions = new


def build_nc():
    with _lean_drain():
        return _build_nc_inner()


def _build_nc_inner():
    nc = bass.Bass()
    # a[p, g, i, n] = adjT[(2g+i)*KT + p, n]  (fp8, 0/1 exact)
    a_d = nc.dram_tensor("a", [KT, NKP, 2, NP], FP8, kind="ExternalInput")
    # xe{h,l}[p, g, i, c] = Xhi/lo[(2g+i)*KT + p, c]
    xh_d = nc.dram_tensor("xh", [KT, NKP, 2, XW], FP8, kind="ExternalInput")
    xl_d = nc.dram_tensor("xl", [KT, NKP, 2, XW], FP8, kind="ExternalInput")
    rbi_d = nc.dram_tensor("rbi", [1, NP], BF16, kind="ExternalInput")
    dt_d = nc.dram_tensor("dt", [8, T, NP], BF16, kind="ExternalInput")
    pt_d = nc.dram_tensor("pt", [8, T, NP], BF16, kind="ExternalInput")
    # w1: [24, 96] — per t, rows 0:8 prev-block, 8:16 raw, 16:24 agg
    w1_d = nc.dram_tensor("w1", [24, DIM], BF16, kind="ExternalInput")
    # wcomb: [8, T, 104] — cols 0:96 wf block(t), cols 96:104 w2s block(t)
    wcomb_d = nc.dram_tensor("wcomb", [8, T * 104], BF16,
                             kind="ExternalInput")
    out_d = nc.dram_tensor("out", [DIM, NP], BF16, kind="ExternalOutput")

    with TileContext(nc) as tc:
        with (
            tc.tile_pool(name="const", bufs=1) as cpool,
            tc.tile_pool(name="pagg", bufs=2, space="PSUM") as pagg,
            tc.tile_pool(name="prb", bufs=1, space="PSUM") as prb,
            tc.tile_pool(name="pp1", bufs=3, space="PSUM") as pp1,
            tc.tile_pool(name="pcm", bufs=2, space="PSUM") as pcm,
        ):
            # Both HWDGE rings stream in parallel: tiny rbi leads SP (it
            # gates the rb matmuls at the head of the PE queue); a-slices
            # alternate across the rings in k order so phase-1 chases a
            # double-rate stream. xl rides mid-ACT (only the lo pass needs
            # it, by which time it has landed).
            rbi_t = cpool.tile([1, NP], BF16)
            nc.sync.dma_start(out=rbi_t, in_=rbi_d[:, :])
            # xh split: the first matmuls gate on just pairs 0-3 (98KB)
            # instead of the full 0.49MB, starting phase 1 ~4us earlier.
            xha_t = cpool.tile([KT, PSL, 2, XW], FP8)
            nc.sync.dma_start(out=xha_t, in_=xh_d[:, 0:PSL, :, :])
            xhb_t = cpool.tile([KT, NKP - PSL, 2, XW], FP8)
            w1_t = cpool.tile([24, DIM], BF16)
            nc.scalar.dma_start(out=w1_t, in_=w1_d[:, :])
            wcomb_t = cpool.tile([8, T * 104], BF16)
            nc.scalar.dma_start(out=wcomb_t, in_=wcomb_d[:, :])
            # dag: per-t matmul rhs [24, T, NP] — rows 0:8 prev state
            # (written by the chain relu), 8:16 dataT, 16:24 scaled agg
            dag_t = cpool.tile([24, T, NP], BF16)
            nc.scalar.dma_start(out=dag_t[8:16, :, :], in_=dt_d[:, :, :])
            nc.vector.memset(dag_t[0:8, 0, :], 0.0)
            # Ring balance: ACT runs at ~1/3 of SP's rate, so it carries only
            # xl (early, gates the first lo matmuls) and s3; the other four
            # a-slices ride SP. This drops the ACT ring's critical load from
            # 2.0MB (~27us, the real phase-1 limiter) to ~1.3MB.
            xl_t = cpool.tile([KT, NKP, 2, XW], FP8)
            nc.scalar.dma_start(out=xl_t, in_=xl_d[:, :, :, :])
            a_tiles = []
            slice_rings = [nc.sync, nc.sync, nc.sync, nc.sync, nc.scalar]
            for s in range(NSL):
                a_t = cpool.tile([KT, PSL, 2, NP], FP8, name=f"a{s}")
                slice_rings[s].dma_start(
                    out=a_t, in_=a_d[:, s * PSL:(s + 1) * PSL, :, :])
                a_tiles.append(a_t)
                if s == 0:
                    nc.sync.dma_start(out=xhb_t,
                                      in_=xh_d[:, PSL:NKP, :, :])
            pt_t = cpool.tile([8, T, NP], BF16)
            nc.scalar.dma_start(out=pt_t, in_=pt_d[:, :, :])

            ones_t = cpool.tile([1, DIM], BF16)
            nc.vector.memset(ones_t, 1.0)
            h2_t = cpool.tile([8, T, NP], BF16)
            aggs_t = cpool.tile([DIM, NP], BF16)
            outt_t = cpool.tile([DIM, NP], BF16)

            aggp_t = [pagg.tile([XW, NH], F32, tag="aggp", name=f"aggp{h}")
                      for h in range(2)]
            rb_t = [prb.tile([DIM, NH], F32, tag="rb", name=f"rb{h}")
                    for h in range(2)]
            pcombs = [pcm.tile([104, NH], F32, tag="pcm", name=f"pcomb{h}")
                      for h in range(2)]

            # inv-degree broadcast to 96 partitions — no phase-1 dependency,
            # emitted first so the PE queue clears it early. Copied to SBUF
            # via DVE max(x,0) (values positive; DVE reads at most one PSUM
            # operand per instruction, and the Scalar queue is busy issuing
            # DMAs at kernel start).
            rbs_t = cpool.tile([DIM, NP], BF16)
            for h in range(2):
                nc.tensor.matmul(rb_t[h], ones_t,
                                 rbi_t[:, h * NH:(h + 1) * NH],
                                 start=True, stop=True, skip_group_check=True)
                nc.vector.tensor_scalar_max(
                    rbs_t[:, h * NH:(h + 1) * NH], rb_t[h], 0.0)

            # phase 1: aggT[96, NH] += X{hi,lo}^T @ adjT_shard, fp8 DoubleRow
            # (2 k-tiles per mm). hi/lo interleave per SLICE so each slice's
            # lo matmuls run right after its hi ones — no lo-pass PE tail
            # after the stream ends.
            rings = [nc.sync, nc.gpsimd, nc.scalar]
            for s in range(NSL):
                a_t = a_tiles[s]
                for xi in range(2):
                    for j in range(PSL):
                        g = s * PSL + j
                        if xi == 1:
                            x_ap = xl_t[:, g, :, :]
                        elif s == 0:
                            x_ap = xha_t[:, j, :, :]
                        else:
                            x_ap = xhb_t[:, g - PSL, :, :]
                        for h in range(2):
                            nc.tensor.matmul(
                                aggp_t[h], x_ap,
                                a_t[:, j, :, h * NH:(h + 1) * NH],
                                start=(g == 0 and xi == 0),
                                stop=(g == NKP - 1 and xi == 1),
                                perf_mode=DR, skip_group_check=True)

            # transition: scale by inv-degree, then scatter per-t slabs into
            # dag full-width in t order. Slabs 0 and 1 get their own scale
            # ops so the chain's first steps start ~2us earlier; the rest
            # are scaled in one bulk op (DVE cost is free-dim-bound, so a
            # [80, x] op costs the same as an [8, x] one).
            def scale(r0, r1):
                for h in range(2):
                    cs = slice(h * NH, (h + 1) * NH)
                    nc.vector.tensor_mul(aggs_t[r0:r1, cs],
                                         aggp_t[h][r0:r1, :],
                                         rbs_t[r0:r1, cs])

            def scatter(t):
                rings[t % 3].dma_start(out=dag_t[16:24, t, :],
                                       in_=aggs_t[t * 8:(t + 1) * 8, :])

            scale(0, 32)
            scatter(0)
            scatter(1)
            scale(32, 64)
            scatter(2)
            scatter(3)
            scale(64, DIM)
            for t in range(4, T):
                scatter(t)

            # phase 2: sequential t-chain, both node halves interleaved.
            # Per (h, t): p1 = w1[t]^T @ dag[:, t]; h2 = relu(p1)+pos (DVE);
            # pcomb += wcomb[t]^T @ h2 (rows 0:96 final acc, 96:104 prev acc,
            # read mid-group by the prev relu). PE order p1(0), p1(1),
            # pcomb(0), pcomb(1) fills the p1->STT bubble with useful work;
            # prev relus split across ScalarE (h0) and DVE (h1).
            def chain_step(t):
                r8 = slice(t * 8, t * 8 + 8)
                p1s = []
                for h in range(2):
                    cs = slice(h * NH, (h + 1) * NH)
                    p1 = pp1.tile([8, NH], F32, tag="p1", name=f"p1_{h}_{t}")
                    nc.tensor.matmul(p1, w1_t[:, r8], dag_t[:, t, cs],
                                     start=True, stop=True)
                    p1s.append(p1)
                for h in range(2):
                    cs = slice(h * NH, (h + 1) * NH)
                    nc.vector.scalar_tensor_tensor(
                        h2_t[:, t, cs], p1s[h], 0.0, pt_t[:, t, cs],
                        op0=mybir.AluOpType.max, op1=mybir.AluOpType.add)
                for h in range(2):
                    cs = slice(h * NH, (h + 1) * NH)
                    nc.tensor.matmul(pcombs[h],
                                     wcomb_t[:, t * 104:(t + 1) * 104],
                                     h2_t[:, t, cs],
                                     start=(t == 0), stop=(t == T - 1),
                                     skip_group_check=True)
                if t < T - 1:
                    nc.scalar.activation(
                        dag_t[0:8, t + 1, 0:NH], pcombs[0][DIM:104, :],
                        mybir.ActivationFunctionType.Relu)
                    nc.vector.tensor_scalar_max(
                        dag_t[0:8, t + 1, NH:NP], pcombs[1][DIM:104, :], 0.0)

            def final(h, pcomb):
                cs = slice(h * NH, (h + 1) * NH)
                if h == 0:
                    nc.scalar.activation(outt_t[:, cs], pcomb[0:DIM, :],
                                         mybir.ActivationFunctionType.Relu)
                else:
                    nc.vector.tensor_scalar_max(outt_t[:, cs],
                                                pcomb[0:DIM, :], 0.0)
                rings[h].dma_start(out=out_d[:, cs], in_=outt_t[:, cs])

            for t in range(T):
                chain_step(t)
            final(0, pcombs[0])
            final(1, pcombs[1])

    split_multi_waits(nc)
    return nc


def prep_in_maps(adj, data, pos, his_W, cur_W, his_weight, cur_weight,
                 final_weight):
    adj = np.asarray(adj, dtype=np.float32)
    data = np.asarray(data, dtype=np.float32)
    pos = np.asarray(pos, dtype=np.float32)
    his_W = np.asarray(his_W, dtype=np.float32)
    cur_W = np.asarray(cur_W, dtype=np.float32)
    his_weight = np.asarray(his_weight, dtype=np.float32)
    cur_weight = np.asarray(cur_weight, dtype=np.float32)
    final_weight = np.asarray(final_weight, dtype=np.float32)

    # X = data rearranged [N, 96] (col = t*8+d); zero-padded to NK rows;
    # hi/lo fp8 split: hi + lo reproduces X to ~bf16 precision.
    X = np.ascontiguousarray(data.transpose(1, 0, 2).reshape(N, DIM))
    Xe = np.zeros((NK, XW), np.float32)
    Xe[:N, :] = X
    Xhi = Xe.astype(FP8_NP)
    Xlo = (Xe - Xhi.astype(np.float32)).astype(FP8_NP)
    # DoubleRow lhsT layout: [KT, NKP, 2, XW], [p, g, i, c] = row (2g+i)*KT+p
    def dr_pack(M):
        return np.ascontiguousarray(
            M.reshape(NKP, 2, KT, M.shape[1]).transpose(2, 0, 1, 3))
    xh_h = dr_pack(Xhi)
    xl_h = dr_pack(Xlo)

    adjT = np.ascontiguousarray(adj.T)
    inv_deg = 1.0 / np.maximum(adj.sum(axis=1), 1.0)  # [N]

    # weight packing (zero-padded block maps, see build_nc layout)
    # w1 [24, 96]: per-t lhsT for the merged p1 matmul over dag rows
    # [prev(8); raw(8); agg(8)]
    w1 = np.zeros((24, DIM), np.float32)
    for t in range(T):
        w1[0:7, t * 8:t * 8 + 7] = his_W[t][:, 21:28].T
        w1[7, t * 8 + 7] = cur_W[t][0, 3]
        w1[8:15, t * 8:t * 8 + 7] = his_W[t][:, 0:7].T
        w1[15, t * 8 + 7] = cur_W[t][0, 0]
        w1[16:23, t * 8:t * 8 + 7] = his_W[t][:, 7:14].T
        w1[23, t * 8 + 7] = cur_W[t][0, 1]
    # w2s[d, 8t'+o] = prev-update weight from h(t') feature d to output o
    w2 = np.zeros((8, DIM), np.float32)
    for tp in range(T):
        w2[0:7, tp * 8:tp * 8 + 7] = his_weight[:, 7 * tp:7 * tp + 7].T
        w2[7, tp * 8 + 7] = cur_weight[0, tp]
    # interleaved feature (8t+d) -> reference feature (7t+d | 84+t)
    f_ref = np.array([7 * t + d if d < 7 else 84 + t
                      for t in range(T) for d in range(8)])
    wf96 = final_weight[:, f_ref].T  # [96 (8t+d), 96 (out)]
    wf = np.ascontiguousarray(
        wf96.reshape(T, 8, DIM).transpose(1, 0, 2).reshape(8, T * DIM))
    # wcomb [8, T*104]: per t, cols 0:96 = wf block(t), cols 96:104 = w2s(t)
    wcomb = np.zeros((8, T, 104), np.float32)
    for t in range(T):
        wcomb[:, t, 0:DIM] = wf[:, t * DIM:(t + 1) * DIM]
        wcomb[:, t, DIM:104] = w2[:, t * 8:(t + 1) * 8]
    wcomb = np.ascontiguousarray(wcomb.reshape(8, T * 104))

    in_maps = []
    for c in range(NCORES):
        c0 = c * NPC
        ac = np.zeros((NK, NP), np.float32)
        ac[:N, :NPC] = adjT[:, c0:c0 + NPC]
        # a[p, g, i, n] = ac[(2g+i)*KT + p, n]
        ah = np.ascontiguousarray(
            ac.reshape(NKP, 2, KT, NP).transpose(2, 0, 1, 3)).astype(FP8_NP)
        rbi = np.zeros((1, NP), np.float32)
        rbi[0, :NPC] = inv_deg[c0:c0 + NPC]
        dtc = np.zeros((8, T, NP), np.float32)
        dtc[:, :, :NPC] = data[:, c0:c0 + NPC, :].transpose(2, 0, 1)
        ptc = np.zeros((8, T, NP), np.float32)
        ptc[:, :, :NPC] = pos[:, c0:c0 + NPC, :].transpose(2, 0, 1)
        in_maps.append({
            "a": ah, "xh": xh_h, "xl": xl_h,
            "rbi": rbi.astype(BF16_NP),
            "dt": dtc.astype(BF16_NP), "pt": ptc.astype(BF16_NP),
            "w1": w1.astype(BF16_NP), "wcomb": wcomb.astype(BF16_NP),
        })
    return in_maps


def assemble(results):
    out = np.empty((N, DIM), np.float32)
    for c in range(NCORES):
        out[c * NPC:(c + 1) * NPC, :] = \
            results[c]["out"][:, :NPC].T.astype(np.float32)
    return out


_NC_CACHE = None


def get_nc():
    global _NC_CACHE
    if _NC_CACHE is None:
        _NC_CACHE = build_nc()
    return _NC_CACHE


def run_spmd(in_maps, **kwargs):
    nc = get_nc()
    return bass_utils.run_bass_kernel_spmd(
        nc, in_maps, list(range(NCORES)), **kwargs)


def kernel(**inputs):
    in_maps = prep_in_maps(**inputs)
    res = run_spmd(in_maps)
    return assemble(res.results)
